# revision 2
# baseline (speedup 1.0000x reference)
"""Locally-connected layer (3x3, stride 1) on 8 Trainium2 NeuronCores.

Shapes (hardcoded):
  x      [B=32, C=96, H=32, W=32]  fp32
  weight [P=900, O=96, K=864]      fp32   (K = C*3*3, channel-major (c,kh,kw))
  bias   [P=900, O=96]             fp32
  out    [B=32, O=96, 30, 30]      fp32

Strategy:
  - Shard the 30x30 patch grid by output rows, padded to 32 rows -> 4 rows
    (120 patches) per core.  One SPMD program on all 8 cores.
  - Per patch, contract K=864 as 9 accumulating matmuls of K=C=96:
    out[b,o] += x[:, i+di, j+dj, b].T @ W[p, dd][:, o].
  - Stationary (lhsT) = x columns [96c, 32b] read in place from an SBUF-resident
    x slice laid out [c, h, w, b]; moving (rhs) = per-patch weight [96c, 96o].
  - Groups of 4 (or 3) adjacent patches are col-tiled onto the 128-wide PE
    array via tile_position=(0, 32u) so their matmuls run concurrently.
  - Weights are streamed from HBM in 8 half-row chunks (15 patches, ~5 MB),
    double buffered; host pre-transposes weight to [c, p, dd, o] so each
    chunk is contiguous per partition.
"""

import os

import numpy as np

B, C, O, H, W = 32, 96, 96, 32, 32
OH = OW = 30
NCORES = 8
ROWS_PER_CORE = 4            # padded 32 output rows / 8 cores
P_CORE = ROWS_PER_CORE * OW  # 120 patches per core
XROWS = ROWS_PER_CORE + 2    # input rows needed per core (halo)
CH = 15                      # patches per weight chunk (half output row)
NCHUNK = P_CORE // CH        # 8

LAST_RESULT = None           # BassKernelResults of the most recent run
_NC_CACHE = {}
KERNEL_KW = {}               # _build_bass kwargs for the kernel() path


def _chunk_groups(cp):
    """Split a chunk of cp consecutive patches into col-tile groups of <=4."""
    groups, j = [], 0
    while j < cp:
        g = min(4, cp - j)
        if cp - j == 5:      # avoid a trailing group of 1
            g = 3
        groups.append((j, g))
        j += g
    return groups


def _build_bass(reps=1, with_wdma=True, with_mm=True, with_out=True,
                row_out=False, chunk_patches=CH, wbufs=2, f32r=False,
                alt_ring=False):
    import concourse.bass as bass
    import concourse.mybir as mybir
    import concourse.tile as tile
    from concourse import bacc

    cp = chunk_patches
    assert OW % cp == 0
    cpr = OW // cp                       # chunks per row
    groups = _chunk_groups(cp)
    n_groups = ROWS_PER_CORE * cpr * len(groups)
    otw = n_groups * O

    f32 = mybir.dt.float32
    mm_dt = mybir.dt.float32r if f32r else f32
    nc = bacc.Bacc("TRN2", target_bir_lowering=False, debug=False,
                   num_devices=NCORES)
    xsd = nc.dram_tensor("xs", [C, XROWS, W, B], f32, kind="ExternalInput")
    wsd = nc.dram_tensor("ws", [C, P_CORE, 9, O], f32, kind="ExternalInput")
    od = nc.dram_tensor("out", [128, otw], f32, kind="ExternalOutput")

    def load(dst, src):
        if f32r:
            nc.gpsimd.dma_start(dst, src)   # SWDGE: casts f32 -> f32r
        else:
            nc.sync.dma_start(dst, src)

    def load_x(dst, src):
        # x + output ride the SWDGE ring so the HWDGE ring is dedicated to
        # the weight stream (the critical path)
        nc.gpsimd.dma_start(dst, src)

    with tile.TileContext(nc) as tc:
        with (
            tc.tile_pool(name="xp", bufs=1) as xp,
            tc.tile_pool(name="wp", bufs=wbufs) as wp,
            tc.tile_pool(name="op", bufs=1) as op,
            tc.tile_pool(name="pp", bufs=8, space=bass.MemorySpace.PSUM) as pp,
        ):
            xt = xp.tile([C, XROWS, W, B], mm_dt)
            load_x(xt[:], xsd[:])
            ot = op.tile([128, otw], f32)

            wt_fixed = None
            if not with_wdma:
                # mm-only probe: one persistent weight tile, loaded once
                wt_fixed = xp.tile([C, cp, 9, O], mm_dt)
                load(wt_fixed[:], wsd[:, 0:cp, :, :])
            if not with_mm and with_out:
                nc.vector.memset(ot[:], 0.0)

            for _rep in range(reps):
                for ch in range(ROWS_PER_CORE * cpr):
                    li, ci = ch // cpr, ch % cpr
                    if with_wdma:
                        wt = wp.tile([C, cp, 9, O], mm_dt)
                        src = wsd[:, ch * cp:(ch + 1) * cp, :, :]
                        if alt_ring and ch % 2 == 1:
                            nc.scalar.dma_start(wt[:], src)
                        else:
                            load(wt[:], src)
                    else:
                        wt = wt_fixed
                    if with_mm:
                        for gi, (jo, gsz) in enumerate(groups):
                            j0 = ci * cp + jo
                            ps = pp.tile([128, O], f32)
                            for dd in range(9):
                                di, dj = dd // 3, dd % 3
                                for u in range(gsz):
                                    nc.tensor.matmul(
                                        ps[32 * u:32 * (u + 1), :],
                                        xt[:, li + di, j0 + u + dj, :],
                                        wt[:, jo + u, dd, :],
                                        start=(dd == 0),
                                        stop=(dd == 8),
                                        tile_position=(0, 32 * u),
                                    )
                            g = (li * cpr + ci) * len(groups) + gi
                            nc.vector.tensor_copy(
                                ot[0:32 * gsz, g * O:(g + 1) * O],
                                ps[0:32 * gsz, :])
                    if with_out and row_out and ci == cpr - 1:
                        gw = cpr * len(groups) * O
                        nc.gpsimd.dma_start(od[:, li * gw:(li + 1) * gw],
                                            ot[:, li * gw:(li + 1) * gw])
                if with_out and not row_out:
                    nc.sync.dma_start(od[:], ot[:])
    nc.compile()
    return nc


def _get_nc():
    key = tuple(sorted(KERNEL_KW.items()))
    if key not in _NC_CACHE:
        _NC_CACHE[key] = _build_bass(**KERNEL_KW)
    return _NC_CACHE[key]


def _build_tiny():
    """Same-I/O trivial kernel for marginal-cost timing (see bench.py)."""
    import concourse.mybir as mybir
    import concourse.tile as tile
    from concourse import bacc

    cp = KERNEL_KW.get("chunk_patches", CH)
    cpr = OW // cp
    n_groups = ROWS_PER_CORE * cpr * len(_chunk_groups(cp))
    otw = n_groups * O

    f32 = mybir.dt.float32
    nc = bacc.Bacc("TRN2", target_bir_lowering=False, debug=False,
                   num_devices=NCORES)
    xsd = nc.dram_tensor("xs", [C, XROWS, W, B], f32, kind="ExternalInput")
    wsd = nc.dram_tensor("ws", [C, P_CORE, 9, O], f32, kind="ExternalInput")
    od = nc.dram_tensor("out", [128, otw], f32, kind="ExternalOutput")
    with tile.TileContext(nc) as tc:
        with tc.tile_pool(name="tp", bufs=1) as tp:
            xt = tp.tile([C, B], f32)
            nc.sync.dma_start(xt[:], xsd[:, 0, 0, :])
            wt = tp.tile([C, O], f32)
            nc.sync.dma_start(wt[:], wsd[:, 0, 0, :])
            ot = tp.tile([128, 8], f32)
            nc.vector.memset(ot[:], 0.0)
            nc.sync.dma_start(od[:, 0:8], ot[:])
    nc.compile()
    return nc


def _prep_in_maps(x, weight):
    # weight [900, O, C*3*3] -> [C, P_pad=960, dd, O]
    w5 = weight.reshape(OH * OW, O, C, 3, 3)
    wt = w5.transpose(2, 0, 3, 4, 1).reshape(C, OH * OW, 9, O)
    wpad = np.zeros((C, NCORES * P_CORE, 9, O), dtype=np.float32)
    wpad[:, :OH * OW] = wt

    # x [B, C, H, W] -> [C, H_pad=34, W, B]
    xt = x.transpose(1, 2, 3, 0)
    xpad = np.zeros((C, H + 2, W, B), dtype=np.float32)
    xpad[:, :H] = xt

    in_maps = []
    for c in range(NCORES):
        in_maps.append({
            "xs": np.ascontiguousarray(
                xpad[:, ROWS_PER_CORE * c:ROWS_PER_CORE * c + XROWS]),
            "ws": np.ascontiguousarray(
                wpad[:, P_CORE * c:P_CORE * (c + 1)]),
        })
    return in_maps


def kernel(x, weight, bias):
    global LAST_RESULT
    from concourse.bass_utils import run_bass_kernel_spmd

    x = np.asarray(x, dtype=np.float32)
    weight = np.asarray(weight, dtype=np.float32)
    bias = np.asarray(bias, dtype=np.float32)

    in_maps = _prep_in_maps(x, weight)
    nc = _get_nc()
    LAST_RESULT = run_bass_kernel_spmd(
        nc, in_maps, core_ids=list(range(NCORES)), trace=False)

    # ---- gather: per-core [128, n_groups*96] -> full [B, O, 30, 30] ----
    groups = _chunk_groups(CH)
    cpr = OW // CH
    n_groups = ROWS_PER_CORE * cpr * len(groups)
    out = np.zeros((B, O, OH, OW), dtype=np.float32)
    for c in range(NCORES):
        oc = LAST_RESULT.results[c]["out"].reshape(4, 32, n_groups, O)
        for li in range(ROWS_PER_CORE):
            i = ROWS_PER_CORE * c + li
            if i >= OH:
                continue
            for ci in range(cpr):
                for gi, (jo, gsz) in enumerate(groups):
                    j0 = ci * CH + jo
                    g = (li * cpr + ci) * len(groups) + gi
                    blk = oc[:gsz, :, g, :]            # [u, b, o]
                    out[:, :, i, j0:j0 + gsz] = blk.transpose(1, 2, 0)
    out += bias.reshape(OH, OW, O).transpose(2, 0, 1)[None]
    return out



# revision 21
# speedup vs baseline: 1.6546x; 1.6546x over previous
"""Locally-connected layer (3x3, stride 1) on 8 Trainium2 NeuronCores.

Shapes (hardcoded):
  x      [B=32, C=96, H=32, W=32]  fp32
  weight [P=900, O=96, K=864]      fp32   (K = C*3*3, channel-major (c,kh,kw))
  bias   [P=900, O=96]             fp32
  out    [B=32, O=96, 30, 30]      fp32

Strategy:
  - Shard the 30x30 patch grid by output rows, padded to 32 rows -> 4 rows
    (120 patches) per core.  One SPMD program on all 8 cores.
  - The kernel streams every weight element once per call, so x and weight
    are pre-cast to bf16 on the host (rel-err ~3e-3 << 2e-2 budget), halving
    HBM traffic, and the output is written back as bf16.
  - Patches are processed in groups of 4 (or 3) adjacent columns.  Per
    (group, dd) ONE fused matmul computes the block-diagonal product:
    stationary = x[:, li+di, j0+dj : j0+dj+4, :] ([96c, 128] = 4 patch
    columns x 32 batch), moving = the 4 patches' weights for that dd
    ([96c, 384]); out is [128, 384] in PSUM of which the 4 diagonal
    [32, 96] blocks are the real per-patch outputs.  This quarters the
    PE instruction count at identical PE-array cycles; off-diagonal
    blocks are never read.
  - PSUM->SBUF diagonal copies alternate between the DVE and ACT engines.
  - Weights stream from HBM on the HWDGE ring in per-row chunks (first and
    last chunk tapered small so PE start/drain overlaps the stream), with
    wbufs-deep buffering; x rides the SWDGE ring in two pieces.
  - Host pre-transposes weight to [c, li, dd, j, o] so each chunk is
    contiguous per partition and each fused moving operand [96, 4*96] is
    contiguous.
"""

import numpy as np

B, C, O, H, W = 32, 96, 96, 32, 32
CP = 128                     # C padded to 128 partitions (faster DMA)
OH = OW = 30
NCORES = 8
ROWS_PER_CORE = 4            # padded 32 output rows / 8 cores
P_CORE = ROWS_PER_CORE * OW  # 120 patches per core
XROWS = ROWS_PER_CORE + 2    # input rows needed per core (halo)
CH = 15                      # patches per weight chunk (half output row)

LAST_RESULT = None           # BassKernelResults of the most recent run
_NC_CACHE = {}
KERNEL_KW = {}               # _build_bass kwargs for the kernel() path


def _chunk_groups(cp):
    """Split a span of cp consecutive patches into col groups of <=4."""
    groups, j = [], 0
    while j < cp:
        g = min(4, cp - j)
        if cp - j == 5:      # avoid a trailing group of 1
            g = 3
        groups.append((j, g))
        j += g
    return groups


def _row_groups(cp):
    """Per-row group list [(j0, gsz), ...] fixed by the chunk width cp."""
    out = []
    for cj in range(0, OW, cp):
        out.extend(
            (cj + jo, gsz) for jo, gsz in _chunk_groups(min(cp, OW - cj)))
    return out


def _out_width(cp):
    return ROWS_PER_CORE * len(_row_groups(cp)) * O


def _chunk_schedule(cp, taper, gdma=True, gpp=1):
    """Weight-chunk issue order: list of (li, [(g, j0, gsz), ...]).

    gdma=True: one DMA piece per `gpp` patch groups (fine-grained pipeline).
    """
    rg = _row_groups(cp)
    ng = len(rg)
    per_chunk = gpp if gdma else len(_chunk_groups(cp))
    sched = []
    for li in range(ROWS_PER_CORE):
        row = [(li * ng + gi, j0, gsz) for gi, (j0, gsz) in enumerate(rg)]
        pieces = [row[k:k + per_chunk] for k in range(0, ng, per_chunk)]
        if taper and not gdma and li == 0:
            first = pieces[0]
            pieces = [first[:1], first[1:]] + pieces[1:]
        if taper and not gdma and li == ROWS_PER_CORE - 1:
            last = pieces[-1]
            pieces = pieces[:-1] + [last[:-1], last[-1:]]
        sched.extend((li, p) for p in pieces if p)
    return sched


def _build_bass(reps=1, with_wdma=True, with_mm=True, with_out=True,
                row_out=False, chunk_patches=CH, wbufs=5, bf16=True,
                out_bf16=True, fused=True, taper=True, split_x=True,
                gdma=True, gpp=2, x_on_sync=True, copy_engines=1,
                alt_ring=True):
    import concourse.bass as bass
    import concourse.mybir as mybir
    import concourse.tile as tile
    from concourse import bacc

    cp = chunk_patches
    otw = _out_width(cp)
    sched = _chunk_schedule(cp, taper, gdma, gpp)

    f32 = mybir.dt.float32
    mm_dt = mybir.dt.bfloat16 if bf16 else f32
    out_dt = mybir.dt.bfloat16 if out_bf16 else f32
    nc = bacc.Bacc("TRN2", target_bir_lowering=False, debug=False,
                   num_devices=NCORES)
    xsd = nc.dram_tensor("xs", [CP, XROWS, W, B], mm_dt, kind="ExternalInput")
    # weight layout: [c, li, j, dd, o] -> chunk slices contiguous/partition
    wsd = nc.dram_tensor("ws", [CP, ROWS_PER_CORE, OW, 9, O], mm_dt,
                         kind="ExternalInput")
    od = nc.dram_tensor("out", [128, otw], out_dt, kind="ExternalOutput")

    def load(dst, src):
        nc.sync.dma_start(dst, src)

    def load_x(dst, src):
        # x + output ride the second HWDGE ring (scalar) so the sync ring
        # is dedicated to the weight stream (the critical path)
        nc.scalar.dma_start(dst, src)

    with tile.TileContext(nc) as tc:
        with (
            tc.tile_pool(name="xp", bufs=1) as xp,
            tc.tile_pool(name="wp", bufs=wbufs) as wp,
            tc.tile_pool(name="op", bufs=1) as op,
            tc.tile_pool(name="pp", bufs=8, space=bass.MemorySpace.PSUM) as pp,
        ):
            xt = xp.tile([CP, XROWS, W, B], mm_dt)
            xload = load if x_on_sync else load_x
            if split_x:
                xload(xt[:, 0:3], xsd[:, 0:3])   # rows for li=0 first
            else:
                xload(xt[:], xsd[:])
            ot = op.tile([128, otw], out_dt)

            wt_fixed = None
            if not with_wdma:
                # mm-only probe: one persistent max-width weight tile
                wt_fixed = xp.tile([CP, cp, 9, O], mm_dt)
                load(wt_fixed[:], wsd[:, 0, 0:cp, :, :])
            if not with_mm and with_out:
                nc.vector.memset(ot[:], 0.0)

            for _rep in range(reps):
                x2_issued = not split_x
                for pi, (li, piece) in enumerate(sched):
                    if not x2_issued and (li >= 1 or pi >= 4):
                        # rest of x once row 0's first weights are in flight
                        xload(xt[:, 3:XROWS], xsd[:, 3:XROWS])
                        x2_issued = True
                    jlo = piece[0][1]
                    jhi = piece[-1][1] + piece[-1][2]
                    width = jhi - jlo
                    if with_wdma:
                        wt = wp.tile([CP, width, 9, O], mm_dt, name="wt")
                        src = wsd[:, li, jlo:jhi, :, :]
                        if alt_ring and pi % 2 == 1:
                            nc.scalar.dma_start(wt[:], src)
                        else:
                            load(wt[:], src)
                        woff = jlo
                    else:
                        wt = wt_fixed
                        woff = piece[0][1]  # reuse fixed tile, any offset
                    if not with_mm:
                        continue
                    for g, j0, gsz in piece:
                        jo = min(j0 - woff, cp - gsz) if not with_wdma \
                            else j0 - woff
                        ps = pp.tile([128, 384], mybir.dt.float32, name="ps")
                        if fused:
                            for dd in range(9):
                                di, dj = dd // 3, dd % 3
                                nc.tensor.matmul(
                                    ps[0:32 * gsz, 0:96 * gsz],
                                    xt[:, li + di, j0 + dj:j0 + dj + gsz, :],
                                    wt[:, jo:jo + gsz, dd, :],
                                    start=(dd == 0),
                                    stop=(dd == 8),
                                )
                            for u in range(gsz):
                                dst = ot[32 * u:32 * (u + 1),
                                         g * O:(g + 1) * O]
                                srcp = ps[32 * u:32 * (u + 1),
                                          96 * u:96 * (u + 1)]
                                if copy_engines == 1 or u % 2 == 0:
                                    nc.vector.tensor_copy(dst, srcp)
                                else:
                                    nc.scalar.copy(dst, srcp)
                        else:
                            for dd in range(9):
                                di, dj = dd // 3, dd % 3
                                for u in range(gsz):
                                    nc.tensor.matmul(
                                        ps[32 * u:32 * (u + 1), 0:O],
                                        xt[:, li + di, j0 + u + dj, :],
                                        wt[:, jo + u, dd, :],
                                        start=(dd == 0),
                                        stop=(dd == 8),
                                        tile_position=(0, 32 * u),
                                    )
                            nc.vector.tensor_copy(
                                ot[0:32 * gsz, g * O:(g + 1) * O],
                                ps[0:32 * gsz, 0:O])
                    if with_out and row_out and jhi == OW:
                        ng = len(_row_groups(cp))
                        gw = ng * O
                        nc.scalar.dma_start(od[:, li * gw:(li + 1) * gw],
                                            ot[:, li * gw:(li + 1) * gw])
                if with_out and not row_out:
                    nc.sync.dma_start(od[:], ot[:])
    nc.compile()
    return nc


def _get_nc():
    key = tuple(sorted(KERNEL_KW.items()))
    if key not in _NC_CACHE:
        _NC_CACHE[key] = _build_bass(**KERNEL_KW)
    return _NC_CACHE[key]


def _build_tiny():
    """Same-I/O trivial kernel for marginal-cost timing (see bench.py)."""
    import concourse.mybir as mybir
    import concourse.tile as tile
    from concourse import bacc

    cp = KERNEL_KW.get("chunk_patches", CH)
    bf16 = KERNEL_KW.get("bf16", True)
    out_bf16 = KERNEL_KW.get("out_bf16", True)
    otw = _out_width(cp)

    f32 = mybir.dt.float32
    mm_dt = mybir.dt.bfloat16 if bf16 else f32
    out_dt = mybir.dt.bfloat16 if out_bf16 else f32
    nc = bacc.Bacc("TRN2", target_bir_lowering=False, debug=False,
                   num_devices=NCORES)
    xsd = nc.dram_tensor("xs", [CP, XROWS, W, B], mm_dt, kind="ExternalInput")
    wsd = nc.dram_tensor("ws", [CP, ROWS_PER_CORE, OW, 9, O], mm_dt,
                         kind="ExternalInput")
    od = nc.dram_tensor("out", [128, otw], out_dt, kind="ExternalOutput")
    with tile.TileContext(nc) as tc:
        with tc.tile_pool(name="tp", bufs=1) as tp:
            xt = tp.tile([CP, B], mm_dt)
            nc.sync.dma_start(xt[:], xsd[:, 0, 0, :])
            wt = tp.tile([CP, O], mm_dt)
            nc.sync.dma_start(wt[:], wsd[:, 0, 0, 0, :])
            ot = tp.tile([128, 8], out_dt)
            nc.vector.memset(ot[:], 0.0)
            nc.sync.dma_start(od[:, 0:8], ot[:])
    nc.compile()
    return nc


def _prep_in_maps(x, weight):
    import ml_dtypes

    bf16 = KERNEL_KW.get("bf16", True)
    dt = ml_dtypes.bfloat16 if bf16 else np.float32

    # weight [900, O, C*3*3] -> [C_pad=128, row_pad=32, OW, dd, O]
    w5 = weight.reshape(OH, OW, O, C, 9)
    wt = w5.transpose(3, 0, 1, 4, 2)               # [C, oh, ow, dd, O]
    wpad = np.zeros((CP, NCORES * ROWS_PER_CORE, OW, 9, O), dtype=dt)
    wpad[:C, :OH] = wt.astype(dt)

    # x [B, C, H, W] -> [C_pad=128, H_pad=34, W, B]
    xt = x.transpose(1, 2, 3, 0)
    xpad = np.zeros((CP, H + 2, W, B), dtype=dt)
    xpad[:C, :H] = xt.astype(dt)

    in_maps = []
    for c in range(NCORES):
        in_maps.append({
            "xs": np.ascontiguousarray(
                xpad[:, ROWS_PER_CORE * c:ROWS_PER_CORE * c + XROWS]),
            "ws": np.ascontiguousarray(
                wpad[:, ROWS_PER_CORE * c:ROWS_PER_CORE * (c + 1)]),
        })
    return in_maps


def _gather(per_core_outs, bias):
    """per-core [128, n_groups*96] -> full [B, O, 30, 30] (+bias)."""
    cp = KERNEL_KW.get("chunk_patches", CH)
    rg = _row_groups(cp)
    ng = len(rg)
    out = np.zeros((B, O, OH, OW), dtype=np.float32)
    for c in range(NCORES):
        oc = np.asarray(per_core_outs[c], dtype=np.float32).reshape(
            128, ROWS_PER_CORE * ng, O)
        for li in range(ROWS_PER_CORE):
            i = ROWS_PER_CORE * c + li
            if i >= OH:
                continue
            for gi, (j0, gsz) in enumerate(rg):
                g = li * ng + gi
                blk = oc[:32 * gsz, g, :].reshape(gsz, 32, O)  # [u, b, o]
                out[:, :, i, j0:j0 + gsz] = blk.transpose(1, 2, 0)
    out += bias.reshape(OH, OW, O).transpose(2, 0, 1)[None]
    return out


def kernel(x, weight, bias):
    global LAST_RESULT
    from concourse.bass_utils import run_bass_kernel_spmd

    x = np.asarray(x, dtype=np.float32)
    weight = np.asarray(weight, dtype=np.float32)
    bias = np.asarray(bias, dtype=np.float32)

    in_maps = _prep_in_maps(x, weight)
    nc = _get_nc()
    LAST_RESULT = run_bass_kernel_spmd(
        nc, in_maps, core_ids=list(range(NCORES)), trace=False)

    return _gather(
        [LAST_RESULT.results[c]["out"] for c in range(NCORES)], bias)
